# revision 58
# baseline (speedup 1.0000x reference)
"""LIF readout kernel for Trainium2 (8 NeuronCores, data-parallel over batch).

Reference computation (per element):
    cur[t,b,o] = (x[t] @ W)[b,o] + bias_o + psp          (psp = THRESH/(2T))
    v_t   = DECAY*m_{t-1} + cur_t
    s_t   = (v_t > THRESH)
    m_t   = v_t - s_t*THRESH
    out[b,o] = mean_t s_t

Device strategy per core (B_local = 16 batch rows), ~120us/core measured:
  - Inputs in bf16 (halves DMA bytes; the PE runs bf16 at the same
    1 cycle/row as fp32r and drops fp32r's >=256-col constraint).  PSUM
    accumulates fp32, so only input quantization noise is added
    (rel L2 ~1.1e-2 on the rate output).
  - GEMM in out.T orientation: psum[o_p, (t,b)] = W[:,j*128:+128].T @ x,
    K=2048 via 16 accumulating matmuls per j-tile.  4 t-blocks
    [26,26,26,22] (moving cols S=416/416/416/352) -> 512 matmuls, all
    exec-bound (>=151ns each) so the tensor engine streams at the PE
    roofline (~88us).  The last block is the smallest because its scan is
    the unavoidable serial tail after the final matmul.
  - DMA: a fused head transfer (x0 k-tiles 0-1 + W k-tiles 0-1, one
    straggler-queue penalty instead of two) followed by x0/W chunks
    interleaved by first-demand time; block 0 runs k-outer so matmuls
    start as soon as the head lands (~11.5us) and stream behind the DMA.
  - ~11 throwaway matmuls on junk data warm the PE clock ramp out of the
    way while the head DMA is in flight.
  - ScalarE copies each psum j-tile into an SBUF currents ring laid out
    [o_p, (t, j, b)] so one scan step reads a contiguous [128,128] slice.
  - VectorE scan, 4 ops/step ordered so consecutive DVE instructions are
    never data-dependent (lets the engine pipeline issue with exec,
    ~675ns/step instead of ~870):
        A(t):  sv[t%3] = (sn * -DECAY) + c_t
        Blo(t-1): acc[:, :64]  = (sv' is_gt THETA) + acc[:, :64]
        C(t):  sn = (sv[t%3] is_gt THETA) - sv[t%3]
        Bhi(t-1): acc[:, 64:] = (sv' is_gt THETA) + acc[:, 64:]
    The uniform psp bias is absorbed into THETA and sn's initial value
    (KAPPA trick).  A nonzero per-o bias b folds in with one extra K=1
    accumulating matmul per (j, block).
  - Output: acc [o_p=128, (j,b)=128] spike counts (exact integers in
    bf16) DMA'd raw; host un-permutes and divides by T.
"""
import numpy as np
import ml_dtypes
from contextlib import ExitStack

import concourse.bass as bass
import concourse.tile as tile
from concourse import bacc, mybir
from concourse.bass_utils import run_bass_kernel_spmd

T, B, C, O = 100, 128, 2048, 1000
NCORES = 8
BL = B // NCORES            # 16 batch rows per core
OP = 1024                   # O padded to 8 o-chunks of 128
NJ = OP // 128              # 8
NK = C // 128               # 16
DECAY = 0.9
THRESH = 1.0
PSP = THRESH / (2 * T)      # 0.005
KAPPA = PSP / (1.0 - DECAY)     # 0.05
THETA = THRESH - KAPPA          # 0.95

TBLOCKS = [26, 26, 26, 22]
assert sum(TBLOCKS) == T
# The first matmul needs a single fused head transfer (x0 k0-1 + W k0-1,
# ~0.7 MB); later chunks are sized so each lands just before its k-step is
# demanded (delivery ~0.25-0.3 MB/us vs consumption ~0.52 MB/2.8us).  Every
# transfer pays a ~2us straggler-queue completion penalty, hence one fused
# head instead of two.
HK = 2                                   # k-tiles in the fused head
WSPLITS = [(k, k + 2) for k in range(HK, NK, 2)]
X0SPLITS = [(2, 8), (8, 16)]             # x block-0 chunks after the head

F32 = mybir.dt.float32
BF16 = mybir.dt.bfloat16
BF16_NP = ml_dtypes.bfloat16

_cache: dict = {}

# Set by test harness for profiling; harmless defaults for standalone use.
TRACE = False
TRACE_DIR = None
LAST_RESULTS = None


def _build(use_bias: bool):
    nc = bacc.Bacc("TRN2", target_bir_lowering=False, debug=False)

    S0 = BL * TBLOCKS[0]
    dm_d = nc.dram_tensor("dmy", [128, 32], BF16, kind="ExternalInput")
    hd_d = nc.dram_tensor("hd", [128, HK * S0 + HK * OP], BF16,
                          kind="ExternalInput")
    x0_ds = [nc.dram_tensor(f"xp0{i}", [128, (k1 - k0) * S0], BF16,
                            kind="ExternalInput")
             for i, (k0, k1) in enumerate(X0SPLITS)]
    xr_d = nc.dram_tensor(
        "xpr", [128, NK * BL * (T - TBLOCKS[0])], BF16, kind="ExternalInput")
    w_ds = [nc.dram_tensor(f"wp{i}", [128, (k1 - k0) * OP], BF16,
                           kind="ExternalInput")
            for i, (k0, k1) in enumerate(WSPLITS)]
    if use_bias:
        b_d = nc.dram_tensor("bp", [1, OP], BF16, kind="ExternalInput")
    o_d = nc.dram_tensor("acc_raw", [128, 128], BF16, kind="ExternalOutput")

    with tile.TileContext(nc) as tc, ExitStack() as ctx:
        wpool = ctx.enter_context(tc.tile_pool(name="wpool", bufs=1))
        x0pool = ctx.enter_context(tc.tile_pool(name="x0pool", bufs=1))
        xpool = ctx.enter_context(tc.tile_pool(name="xpool", bufs=2))
        cpool = ctx.enter_context(tc.tile_pool(name="cpool", bufs=2))
        spool = ctx.enter_context(tc.tile_pool(name="spool", bufs=1))
        ppool = ctx.enter_context(tc.tile_pool(name="ppool", bufs=1, space="PSUM"))

        # Fused head (x0 k0-1 + W k0-1) first, then x0/W chunks interleaved
        # by first-demand time.
        head = x0pool.tile([128, HK * S0 + HK * OP], BF16, name="head")
        xt0 = [x0pool.tile([128, (k1 - k0) * S0], BF16, name=f"xt0_{i}")
               for i, (k0, k1) in enumerate(X0SPLITS)]
        wc = [wpool.tile([128, (k1 - k0) * OP], BF16, name=f"wc{i}")
              for i, (k0, k1) in enumerate(WSPLITS)]
        # A tiny dummy transfer absorbs the per-engine DMA cold-start (one
        # queue engine issues its first packet ~2us late), so the head
        # transfer behind it completes without the straggler penalty.
        dmy = wpool.tile([128, 32], BF16, name="dmy")
        nc.sync.dma_start(dmy[:], dm_d[:])
        nc.sync.dma_start(head[:], hd_d[:])
        for kind, i in [("x", 0), ("w", 0), ("w", 1), ("w", 2),
                        ("x", 1), ("w", 3), ("w", 4), ("w", 5), ("w", 6)]:
            if kind == "x":
                nc.sync.dma_start(xt0[i][:], x0_ds[i][:])
            else:
                nc.sync.dma_start(wc[i][:], w_ds[i][:])
        wk = [head[:, HK * S0 + k * OP:HK * S0 + (k + 1) * OP]
              for k in range(HK)]
        for i, (k0, k1) in enumerate(WSPLITS):
            for k in range(k0, k1):
                wk.append(wc[i][:, (k - k0) * OP:(k - k0 + 1) * OP])
        x0src = {k: ("h", k) for k in range(HK)}
        for i, (k0, k1) in enumerate(X0SPLITS):
            for k in range(k0, k1):
                x0src[k] = (i, k - k0)

        def xt0_src(k, S):
            i, kk = x0src[k]
            if i == "h":
                return head[:, kk * S:(kk + 1) * S]
            return xt0[i][:, kk * S:(kk + 1) * S]
        if use_bias:
            bt = wpool.tile([1, OP], BF16, name="bt")
            nc.sync.dma_start(bt[:], b_d[:])
            ones = wpool.tile([1, 512], BF16, name="ones")
            nc.vector.memset(ones[:], 1.0)

        # Pre-warm the PE array during the head-DMA wait: the tensor engine
        # ramps to full clock only after ~3us of continuous execution, so a
        # dozen throwaway matmuls on junk data (no DMA deps, result never
        # read) get the ramp out of the way before the first real matmul.
        junk = wpool.tile([128, 512], BF16, name="junk")
        nc.vector.memset(junk[:], 1.0)
        warm = ppool.tile([128, 512], F32, tag="ps0", name="warm")
        # One accumulation group: start/stop on every matmul would drain the
        # PE pipeline between them and the clock never ramps.
        NWARM = 8
        for i in range(NWARM):
            nc.tensor.matmul(warm[:], junk[:, :128], junk[:],
                             start=(i == 0), stop=(i == NWARM - 1))

        sv = [spool.tile([128, 128], F32, name=f"sv{i}") for i in range(3)]
        sn = spool.tile([128, 128], F32, name="sn")
        # spike counts are integers <= T=100: exact in bf16, and the 2-byte
        # accumulator halves the DVE cost of the acc ops
        acc = spool.tile([128, 128], BF16, name="acc")
        nc.vector.memset(sn[:], KAPPA)
        nc.vector.memset(acc[:], 0.0)

        # Emit matmuls+copies for one t-block; returns the currents ring tile.
        def do_block(bi, coff):
            tb = TBLOCKS[bi]
            S = BL * tb
            cur = cpool.tile([128, tb * 128], F32, tag="cur", name=f"cur{bi}")
            cur3 = cur[:].rearrange("p (t v) -> p t v", v=128)
            ps = [ppool.tile([128, S], F32, tag=f"ps{j}", name=f"ps{bi}_{j}")
                  for j in range(NJ)]

            def finish(j):
                if use_bias:
                    nc.tensor.matmul(
                        ps[j][:], bt[:, j * 128:(j + 1) * 128], ones[:, :S],
                        start=False, stop=True)
                nc.scalar.copy(
                    cur3[:, :, j * BL:(j + 1) * BL],
                    ps[j][:].rearrange("p (t b) -> p t b", b=BL))

            if bi == 0:
                # k-outer: stream as W chunks arrive
                for k in range(NK):
                    xsrc = xt0_src(k, S)
                    for j in range(NJ):
                        nc.tensor.matmul(
                            ps[j][:],
                            wk[k][:, j * 128:(j + 1) * 128],
                            xsrc,
                            start=(k == 0),
                            stop=(k == NK - 1 and not use_bias),
                        )
                for j in range(NJ):
                    finish(j)
            else:
                xt = xpool.tile([128, NK * S], BF16, tag="xt", name=f"xt{bi}")
                nc.sync.dma_start(xt[:], xr_d[:, coff:coff + NK * S])
                for j in range(NJ):
                    for k in range(NK):
                        nc.tensor.matmul(
                            ps[j][:],
                            wk[k][:, j * 128:(j + 1) * 128],
                            xt[:, k * S:(k + 1) * S],
                            start=(k == 0),
                            stop=(k == NK - 1 and not use_bias),
                        )
                    finish(j)
            return cur

        coff = 0
        curs = []
        for bi, tb in enumerate(TBLOCKS):
            curs.append(do_block(bi, coff))
            coff += NK * BL * tb if bi > 0 else 0

        def acc_half(t, lo):
            h = slice(0, 64) if lo else slice(64, 128)
            nc.vector.scalar_tensor_tensor(
                out=acc[:, h], in0=sv[t % 3][:, h], scalar=THETA,
                in1=acc[:, h],
                op0=mybir.AluOpType.is_gt, op1=mybir.AluOpType.add)

        # 4 DVE ops per step, ordered so consecutive ops are independent
        # (ping-ponged sv + acc halves as fillers) -> the engine pipelines
        # issue with exec instead of stalling on each op's writeback.
        t = 0
        for bi, tb in enumerate(TBLOCKS):
            cur = curs[bi]
            for tl in range(tb):
                c_t = cur[:, tl * 128:(tl + 1) * 128]
                nc.vector.scalar_tensor_tensor(
                    out=sv[t % 3][:], in0=sn[:], scalar=-DECAY, in1=c_t,
                    op0=mybir.AluOpType.mult, op1=mybir.AluOpType.add)
                if t > 0:
                    acc_half(t - 1, True)
                if t < T - 1:
                    # sn(T-1) is never read: skip the final C op
                    nc.vector.scalar_tensor_tensor(
                        out=sn[:], in0=sv[t % 3][:], scalar=THETA,
                        in1=sv[t % 3][:],
                        op0=mybir.AluOpType.is_gt,
                        op1=mybir.AluOpType.subtract)
                if t > 0:
                    acc_half(t - 1, False)
                t += 1
        # fire each output half as soon as its accumulator is final, so the
        # DMA trigger overlaps the last scan op instead of following it
        acc_half(T - 1, True)
        nc.sync.dma_start(o_d[:, 0:64], acc[:, 0:64])
        acc_half(T - 1, False)
        nc.sync.dma_start(o_d[:, 64:128], acc[:, 64:128])

    nc.finalize()
    return nc


def _prep_x(x_core: np.ndarray) -> tuple[np.ndarray, np.ndarray]:
    """x_core [T, BL, C] -> (block-0 [128, NK*S0], rest [128, ...]) bf16,
    block-major with (k, m) layout inside each block."""
    xm = np.ascontiguousarray(x_core.reshape(T * BL, C).T)   # [C, M]
    xk = xm.reshape(NK, 128, T * BL)                         # [k, p, m]
    segs = []
    m0 = 0
    for tb in TBLOCKS:
        S = BL * tb
        seg = xk[:, :, m0:m0 + S].transpose(1, 0, 2).reshape(128, NK * S)
        segs.append(seg)
        m0 += S
    S0 = BL * TBLOCKS[0]
    x0head = segs[0][:, :HK * S0]
    x0parts = [
        np.ascontiguousarray(segs[0][:, k0 * S0:k1 * S0]).astype(BF16_NP)
        for (k0, k1) in X0SPLITS
    ]
    xr = np.concatenate(segs[1:], axis=1).astype(BF16_NP)
    return x0head, x0parts, xr


def kernel(x: np.ndarray, W: np.ndarray, b: np.ndarray) -> np.ndarray:
    x = np.asarray(x, dtype=np.float32)
    W = np.asarray(W, dtype=np.float32)
    b = np.asarray(b, dtype=np.float32)
    use_bias = bool(np.any(b != 0.0))

    key = use_bias
    if key not in _cache:
        _cache[key] = _build(use_bias)
    nc = _cache[key]

    Wp = np.zeros((C, OP), np.float32)
    Wp[:, :O] = W
    wprep = np.ascontiguousarray(
        Wp.reshape(NK, 128, OP).transpose(1, 0, 2).reshape(128, NK * OP)
    ).astype(BF16_NP)
    wparts = {
        f"wp{i}": np.ascontiguousarray(wprep[:, k0 * OP:k1 * OP])
        for i, (k0, k1) in enumerate(WSPLITS)
    }
    whead = wprep[:, :HK * OP].astype(np.float32)

    dmy = np.zeros((128, 32), BF16_NP)
    in_maps = []
    for c in range(NCORES):
        x0head, x0parts, xr = _prep_x(x[:, c * BL:(c + 1) * BL, :])
        hd = np.concatenate(
            [x0head.astype(np.float32), whead], axis=1).astype(BF16_NP)
        m = {"dmy": dmy, "hd": hd, "xpr": xr, **wparts}
        for i, part in enumerate(x0parts):
            m[f"xp0{i}"] = part
        if use_bias:
            bp = np.zeros((1, OP), np.float32)
            bp[0, :O] = b
            m["bp"] = bp.astype(BF16_NP)
        in_maps.append(m)

    global LAST_RESULTS
    if TRACE:
        res = run_bass_kernel_spmd(
            nc, in_maps, list(range(NCORES)), trace=True,
            trace_cores=list(range(NCORES)), tmpdir=TRACE_DIR)
    else:
        res = run_bass_kernel_spmd(nc, in_maps, list(range(NCORES)))
    LAST_RESULTS = res

    outs = []
    for c in range(NCORES):
        raw = np.asarray(res.results[c]["acc_raw"], dtype=np.float32)
        rate = raw.reshape(128, NJ, BL).transpose(2, 1, 0).reshape(BL, OP)
        outs.append(rate[:, :O] / np.float32(T))
    return np.concatenate(outs, axis=0).astype(np.float32)


# revision 62
# speedup vs baseline: 1.0227x; 1.0227x over previous
"""LIF readout kernel for Trainium2 (8 NeuronCores, data-parallel over batch).

Reference computation (per element):
    cur[t,b,o] = (x[t] @ W)[b,o] + bias_o + psp          (psp = THRESH/(2T))
    v_t   = DECAY*m_{t-1} + cur_t
    s_t   = (v_t > THRESH)
    m_t   = v_t - s_t*THRESH
    out[b,o] = mean_t s_t

Device strategy per core (B_local = 16 batch rows), ~120us/core measured:
  - Inputs in bf16 (halves DMA bytes; the PE runs bf16 at the same
    1 cycle/row as fp32r and drops fp32r's >=256-col constraint).  PSUM
    accumulates fp32, so only input quantization noise is added
    (rel L2 ~1.1e-2 on the rate output).
  - GEMM in out.T orientation: psum[o_p, (t,b)] = W[:,j*128:+128].T @ x,
    K=2048 via 16 accumulating matmuls per j-tile.  4 t-blocks
    [26,26,26,22] (moving cols S=416/416/416/352) -> 512 matmuls, all
    exec-bound (>=151ns each) so the tensor engine streams at the PE
    roofline (~88us).  The last block is the smallest because its scan is
    the unavoidable serial tail after the final matmul.
  - DMA: a fused head transfer (x0 k-tiles 0-1 + W k-tiles 0-1, one
    straggler-queue penalty instead of two) followed by x0/W chunks
    interleaved by first-demand time; block 0 runs k-outer so matmuls
    start as soon as the head lands (~11.5us) and stream behind the DMA.
  - ~11 throwaway matmuls on junk data warm the PE clock ramp out of the
    way while the head DMA is in flight.
  - ScalarE copies each psum j-tile into an SBUF currents ring laid out
    [o_p, (t, j, b)] so one scan step reads a contiguous [128,128] slice.
  - VectorE scan, 4 ops/step ordered so consecutive DVE instructions are
    never data-dependent (lets the engine pipeline issue with exec,
    ~675ns/step instead of ~870):
        A(t):  sv[t%3] = (sn * -DECAY) + c_t
        Blo(t-1): acc[:, :64]  = (sv' is_gt THETA) + acc[:, :64]
        C(t):  sn = (sv[t%3] is_gt THETA) - sv[t%3]
        Bhi(t-1): acc[:, 64:] = (sv' is_gt THETA) + acc[:, 64:]
    The uniform psp bias is absorbed into THETA and sn's initial value
    (KAPPA trick).  A nonzero per-o bias b folds in with one extra K=1
    accumulating matmul per (j, block).
  - Output: acc [o_p=128, (j,b)=128] spike counts (exact integers in
    bf16) DMA'd raw; host un-permutes and divides by T.
"""
import numpy as np
import ml_dtypes
from contextlib import ExitStack

import concourse.bass as bass
import concourse.tile as tile
from concourse import bacc, mybir
from concourse.bass_utils import run_bass_kernel_spmd

T, B, C, O = 100, 128, 2048, 1000
NCORES = 8
BL = B // NCORES            # 16 batch rows per core
OP = 1024                   # O padded to 8 o-chunks of 128
NJ = OP // 128              # 8
NK = C // 128               # 16
DECAY = 0.9
THRESH = 1.0
PSP = THRESH / (2 * T)      # 0.005
KAPPA = PSP / (1.0 - DECAY)     # 0.05
THETA = THRESH - KAPPA          # 0.95

TBLOCKS = [26, 26, 26, 22]
assert sum(TBLOCKS) == T
# The first matmul needs a single fused head transfer (x0 k0-1 + W k0-1,
# ~0.7 MB); later chunks are sized so each lands just before its k-step is
# demanded (delivery ~0.25-0.3 MB/us vs consumption ~0.52 MB/2.8us).  Every
# transfer pays a ~2us straggler-queue completion penalty, hence one fused
# head instead of two.
HK = 2                                   # k-tiles in the fused head
WSPLITS = [(k, k + 2) for k in range(HK, NK, 2)]
X0SPLITS = [(2, 8), (8, 16)]             # x block-0 chunks after the head

F32 = mybir.dt.float32
BF16 = mybir.dt.bfloat16
BF16_NP = ml_dtypes.bfloat16

_cache: dict = {}

# Set by test harness for profiling; harmless defaults for standalone use.
TRACE = False
TRACE_DIR = None
LAST_RESULTS = None


def _build(use_bias: bool):
    nc = bacc.Bacc("TRN2", target_bir_lowering=False, debug=False)

    S0 = BL * TBLOCKS[0]
    hdx_d = nc.dram_tensor("hdx", [128, HK * S0], BF16, kind="ExternalInput")
    hdw_d = nc.dram_tensor("hdw", [128, HK * OP], BF16, kind="ExternalInput")
    x0_ds = [nc.dram_tensor(f"xp0{i}", [128, (k1 - k0) * S0], BF16,
                            kind="ExternalInput")
             for i, (k0, k1) in enumerate(X0SPLITS)]
    xr_d = nc.dram_tensor(
        "xpr", [128, NK * BL * (T - TBLOCKS[0])], BF16, kind="ExternalInput")
    w_ds = [nc.dram_tensor(f"wp{i}", [128, (k1 - k0) * OP], BF16,
                           kind="ExternalInput")
            for i, (k0, k1) in enumerate(WSPLITS)]
    if use_bias:
        b_d = nc.dram_tensor("bp", [1, OP], BF16, kind="ExternalInput")
    o_d = nc.dram_tensor("acc_raw", [128, 128], BF16, kind="ExternalOutput")

    with tile.TileContext(nc) as tc, ExitStack() as ctx:
        wpool = ctx.enter_context(tc.tile_pool(name="wpool", bufs=1))
        x0pool = ctx.enter_context(tc.tile_pool(name="x0pool", bufs=1))
        xpool = ctx.enter_context(tc.tile_pool(name="xpool", bufs=2))
        cpool = ctx.enter_context(tc.tile_pool(name="cpool", bufs=2))
        spool = ctx.enter_context(tc.tile_pool(name="spool", bufs=1))
        ppool = ctx.enter_context(tc.tile_pool(name="ppool", bufs=1, space="PSUM"))

        # Head split across two trigger engines: x-part on sync, W-part on
        # scalar — if their DMA rings differ, the two halves stream
        # concurrently and the first matmul unblocks ~1.5us earlier.
        headx = x0pool.tile([128, HK * S0], BF16, name="headx")
        headw = wpool.tile([128, HK * OP], BF16, name="headw")
        xt0 = [x0pool.tile([128, (k1 - k0) * S0], BF16, name=f"xt0_{i}")
               for i, (k0, k1) in enumerate(X0SPLITS)]
        wc = [wpool.tile([128, (k1 - k0) * OP], BF16, name=f"wc{i}")
              for i, (k0, k1) in enumerate(WSPLITS)]
        nc.sync.dma_start(headx[:], hdx_d[:])
        nc.scalar.dma_start(headw[:], hdw_d[:])
        for kind, i in [("x", 0), ("w", 0), ("w", 1), ("w", 2),
                        ("x", 1), ("w", 3), ("w", 4), ("w", 5), ("w", 6)]:
            if kind == "x":
                nc.sync.dma_start(xt0[i][:], x0_ds[i][:])
            else:
                nc.sync.dma_start(wc[i][:], w_ds[i][:])
        wk = [headw[:, k * OP:(k + 1) * OP] for k in range(HK)]
        for i, (k0, k1) in enumerate(WSPLITS):
            for k in range(k0, k1):
                wk.append(wc[i][:, (k - k0) * OP:(k - k0 + 1) * OP])
        x0src = {k: ("h", k) for k in range(HK)}
        for i, (k0, k1) in enumerate(X0SPLITS):
            for k in range(k0, k1):
                x0src[k] = (i, k - k0)

        def xt0_src(k, S):
            i, kk = x0src[k]
            if i == "h":
                return headx[:, kk * S:(kk + 1) * S]
            return xt0[i][:, kk * S:(kk + 1) * S]
        if use_bias:
            bt = wpool.tile([1, OP], BF16, name="bt")
            nc.sync.dma_start(bt[:], b_d[:])
            ones = wpool.tile([1, 512], BF16, name="ones")
            nc.vector.memset(ones[:], 1.0)

        # Pre-warm the PE array during the head-DMA wait: the tensor engine
        # ramps to full clock only after ~3us of continuous execution, so a
        # dozen throwaway matmuls on junk data (no DMA deps, result never
        # read) get the ramp out of the way before the first real matmul.
        junk = wpool.tile([128, 512], BF16, name="junk")
        nc.vector.memset(junk[:], 1.0)
        warm = ppool.tile([128, 512], F32, tag="ps0", name="warm")
        # One accumulation group: start/stop on every matmul would drain the
        # PE pipeline between them and the clock never ramps.
        NWARM = 11
        for i in range(NWARM):
            nc.tensor.matmul(warm[:], junk[:, :128], junk[:],
                             start=(i == 0), stop=(i == NWARM - 1))

        sv = [spool.tile([128, 128], F32, name=f"sv{i}") for i in range(3)]
        sn = spool.tile([128, 128], F32, name="sn")
        # spike counts are integers <= T=100: exact in bf16, and the 2-byte
        # accumulator halves the DVE cost of the acc ops
        acc = spool.tile([128, 128], BF16, name="acc")
        nc.vector.memset(sn[:], KAPPA)
        nc.vector.memset(acc[:], 0.0)

        # Emit matmuls+copies for one t-block; returns the currents ring tile.
        def do_block(bi, coff):
            tb = TBLOCKS[bi]
            S = BL * tb
            cur = cpool.tile([128, tb * 128], F32, tag="cur", name=f"cur{bi}")
            cur3 = cur[:].rearrange("p (t v) -> p t v", v=128)
            ps = [ppool.tile([128, S], F32, tag=f"ps{j}", name=f"ps{bi}_{j}")
                  for j in range(NJ)]

            def finish(j):
                if use_bias:
                    nc.tensor.matmul(
                        ps[j][:], bt[:, j * 128:(j + 1) * 128], ones[:, :S],
                        start=False, stop=True)
                nc.scalar.copy(
                    cur3[:, :, j * BL:(j + 1) * BL],
                    ps[j][:].rearrange("p (t b) -> p t b", b=BL))

            if bi == 0:
                # k-outer: stream as W chunks arrive
                for k in range(NK):
                    xsrc = xt0_src(k, S)
                    for j in range(NJ):
                        nc.tensor.matmul(
                            ps[j][:],
                            wk[k][:, j * 128:(j + 1) * 128],
                            xsrc,
                            start=(k == 0),
                            stop=(k == NK - 1 and not use_bias),
                        )
                for j in range(NJ):
                    finish(j)
            else:
                xt = xpool.tile([128, NK * S], BF16, tag="xt", name=f"xt{bi}")
                nc.sync.dma_start(xt[:], xr_d[:, coff:coff + NK * S])
                for j in range(NJ):
                    for k in range(NK):
                        nc.tensor.matmul(
                            ps[j][:],
                            wk[k][:, j * 128:(j + 1) * 128],
                            xt[:, k * S:(k + 1) * S],
                            start=(k == 0),
                            stop=(k == NK - 1 and not use_bias),
                        )
                    finish(j)
            return cur

        coff = 0
        curs = []
        for bi, tb in enumerate(TBLOCKS):
            curs.append(do_block(bi, coff))
            coff += NK * BL * tb if bi > 0 else 0

        def acc_half(t, lo):
            h = slice(0, 64) if lo else slice(64, 128)
            nc.vector.scalar_tensor_tensor(
                out=acc[:, h], in0=sv[t % 3][:, h], scalar=THETA,
                in1=acc[:, h],
                op0=mybir.AluOpType.is_gt, op1=mybir.AluOpType.add)

        # 4 DVE ops per step, ordered so consecutive ops are independent
        # (ping-ponged sv + acc halves as fillers) -> the engine pipelines
        # issue with exec instead of stalling on each op's writeback.
        t = 0
        for bi, tb in enumerate(TBLOCKS):
            cur = curs[bi]
            for tl in range(tb):
                c_t = cur[:, tl * 128:(tl + 1) * 128]
                nc.vector.scalar_tensor_tensor(
                    out=sv[t % 3][:], in0=sn[:], scalar=-DECAY, in1=c_t,
                    op0=mybir.AluOpType.mult, op1=mybir.AluOpType.add)
                if t > 0:
                    acc_half(t - 1, True)
                if t < T - 1:
                    # sn(T-1) is never read: skip the final C op
                    nc.vector.scalar_tensor_tensor(
                        out=sn[:], in0=sv[t % 3][:], scalar=THETA,
                        in1=sv[t % 3][:],
                        op0=mybir.AluOpType.is_gt,
                        op1=mybir.AluOpType.subtract)
                if t > 0:
                    acc_half(t - 1, False)
                t += 1
        # fire each output half as soon as its accumulator is final, so the
        # DMA trigger overlaps the last scan op instead of following it
        acc_half(T - 1, True)
        nc.sync.dma_start(o_d[:, 0:64], acc[:, 0:64])
        acc_half(T - 1, False)
        nc.sync.dma_start(o_d[:, 64:128], acc[:, 64:128])

    nc.finalize()
    return nc


def _prep_x(x_core: np.ndarray) -> tuple[np.ndarray, np.ndarray]:
    """x_core [T, BL, C] -> (block-0 [128, NK*S0], rest [128, ...]) bf16,
    block-major with (k, m) layout inside each block."""
    xm = np.ascontiguousarray(x_core.reshape(T * BL, C).T)   # [C, M]
    xk = xm.reshape(NK, 128, T * BL)                         # [k, p, m]
    segs = []
    m0 = 0
    for tb in TBLOCKS:
        S = BL * tb
        seg = xk[:, :, m0:m0 + S].transpose(1, 0, 2).reshape(128, NK * S)
        segs.append(seg)
        m0 += S
    S0 = BL * TBLOCKS[0]
    x0head = segs[0][:, :HK * S0]
    x0parts = [
        np.ascontiguousarray(segs[0][:, k0 * S0:k1 * S0]).astype(BF16_NP)
        for (k0, k1) in X0SPLITS
    ]
    xr = np.concatenate(segs[1:], axis=1).astype(BF16_NP)
    return x0head, x0parts, xr


def kernel(x: np.ndarray, W: np.ndarray, b: np.ndarray) -> np.ndarray:
    x = np.asarray(x, dtype=np.float32)
    W = np.asarray(W, dtype=np.float32)
    b = np.asarray(b, dtype=np.float32)
    use_bias = bool(np.any(b != 0.0))

    key = use_bias
    if key not in _cache:
        _cache[key] = _build(use_bias)
    nc = _cache[key]

    Wp = np.zeros((C, OP), np.float32)
    Wp[:, :O] = W
    wprep = np.ascontiguousarray(
        Wp.reshape(NK, 128, OP).transpose(1, 0, 2).reshape(128, NK * OP)
    ).astype(BF16_NP)
    wparts = {
        f"wp{i}": np.ascontiguousarray(wprep[:, k0 * OP:k1 * OP])
        for i, (k0, k1) in enumerate(WSPLITS)
    }
    whead = wprep[:, :HK * OP].astype(np.float32)

    hdw = np.ascontiguousarray(whead).astype(BF16_NP)
    in_maps = []
    for c in range(NCORES):
        x0head, x0parts, xr = _prep_x(x[:, c * BL:(c + 1) * BL, :])
        m = {"hdx": np.ascontiguousarray(x0head).astype(BF16_NP),
             "hdw": hdw, "xpr": xr, **wparts}
        for i, part in enumerate(x0parts):
            m[f"xp0{i}"] = part
        if use_bias:
            bp = np.zeros((1, OP), np.float32)
            bp[0, :O] = b
            m["bp"] = bp.astype(BF16_NP)
        in_maps.append(m)

    global LAST_RESULTS
    if TRACE:
        res = run_bass_kernel_spmd(
            nc, in_maps, list(range(NCORES)), trace=True,
            trace_cores=list(range(NCORES)), tmpdir=TRACE_DIR)
    else:
        res = run_bass_kernel_spmd(nc, in_maps, list(range(NCORES)))
    LAST_RESULTS = res

    outs = []
    for c in range(NCORES):
        raw = np.asarray(res.results[c]["acc_raw"], dtype=np.float32)
        rate = raw.reshape(128, NJ, BL).transpose(2, 1, 0).reshape(BL, OP)
        outs.append(rate[:, :O] / np.float32(T))
    return np.concatenate(outs, axis=0).astype(np.float32)
